# revision 38
# baseline (speedup 1.0000x reference)
"""Multi-relation GAT layer on 8 Trainium2 NeuronCores.

Sharding: core c handles batch b = c//4 and dst-row quarter i0 = (c%4)*512.
Each core sees all R relations for its (b, i-slice), so the mean over
relations needs no cross-core reduction, and every A element is read by
exactly one core (DMA-minimal).

Host precomputes the cheap dense projections (Wh = H@W, attention dots
es/ed); the device does the heavy O(R*Hh*N^2) masked-softmax attention.

Key reformulation: softmax weights are invariant to any per-row (per-i)
scale, so normalize by exp(-es[i]):

  u[j,i] = exp(lrelu(es[i]+ed[j]) - es[i]) * A[j,i]
         = A[j,i] * max(E1j, E2j * E2i)          (lrelu(s)=max(s, 0.2s))
    E1j = exp(ed[j]), E2j = exp(0.2 ed[j])   per-partition scalars [128,1]
    E2i = exp(-0.8 es[i])                    broadcast-row tensor

The whole masked score tensor is built by ONE fused custom DVE op per
(head, j-tile):   u = Src0 * max(Src1 * C0, C1)
with a hand-authored 2X_1PORT uop program (2 bf16 elems/cycle).  No
scalar-engine exp over N^2, no separate mask multiply.  Aggregation
matmuls contract over j on the partition axis; a column of value R
appended to Wh produces R*rowsum so softmax normalization and the mean
over relations come out of one reciprocal.  The epilogue runs on the
otherwise-idle Scalar (ACT) and GpSimd engines.
"""

import sys

sys.path.insert(0, "/opt/trn_rl_repo")

import numpy as np
import ml_dtypes

R, B, N, D, Hh, hd = 3, 2, 2048, 128, 4, 32
NCORES = 8
NIQ = 4  # i-quarters per batch
IS = N // NIQ  # 512 dst rows per core
NT = N // 128  # 16 j tiles
LN_EPS = 1e-5
HW = Hh * 33  # 132 packed Wh cols per j-tile (32 wh + 1 R-col per head)

_CACHE = {}

_GAT_NAME = "GAT_MASKED_MAX"


def _build_2x_uops(ver):
    """2X_1PORT program for body = Src0 * max(Src1*C0, C1): LO chain in
    stages 0-2, HI chain (SRC_*_HI) in stages 3-5; LO result rides delay
    lane 0 from stage 3; pair written via WR0_LO/WR0_HI."""
    from concourse.dve_uop import (
        DISABLE,
        ENABLE,
        AluInp,
        AluOp,
        DelayInp,
        InpSel,
        OutPath,
        OutSel,
        Trigger,
        UopConfig,
        UopDpConfig,
        _MAX_INPUTS,
        _MAX_LANES,
    )

    PD, PA = DelayInp.PREV_DELAY, DelayInp.PREV_ALU_OUT

    def dp(op, a, b, capture_lane=None):
        delay = [PD] * 6 + [PA] * (_MAX_LANES - 6)
        enable = [1] * 6 + [0] * (_MAX_LANES - 6)
        if capture_lane is not None:
            delay[capture_lane] = PA
        return UopDpConfig(
            op=op,
            alu_src0=a,
            alu_src1=b,
            delay=delay,
            alu_out_enable=1,
            swap_enable=0,
            alu_out_a_enable=0,
            alu_out_b_enable=0,
            delay_enable=enable,
            idx0_sel=0,
            idx1_sel=0,
        )

    A = AluInp
    stages = [
        dp(AluOp.MULTIPLY, A.PREV_DELAY_1, A.PREV_DELAY_2),
        dp(AluOp.MAX, A.PREV_ALU_OUT, A.PREV_DELAY_3),
        dp(AluOp.MULTIPLY, A.PREV_DELAY_0, A.PREV_ALU_OUT),
        dp(AluOp.MULTIPLY, A.PREV_DELAY_5, A.PREV_DELAY_2, capture_lane=0),
        dp(AluOp.MAX, A.PREV_ALU_OUT, A.PREV_DELAY_3),
        dp(AluOp.MULTIPLY, A.PREV_DELAY_4, A.PREV_ALU_OUT),
        dp(AluOp.BYPASS, A.PREV_ALU_OUT, A.PREV_ALU_OUT),
        dp(AluOp.BYPASS, A.PREV_ALU_OUT, A.PREV_ALU_OUT),
    ]

    inp = [InpSel.ZERO] * _MAX_INPUTS
    inp_enable = [DISABLE] * _MAX_INPUTS
    for slot, sel in (
        (1, InpSel.SRC_0),
        (2, InpSel.SRC_1),
        (3, InpSel.CONST_0),
        (4, InpSel.CONST_1),
        (5, InpSel.SRC_0_HI),
        (6, InpSel.SRC_1_HI),
    ):
        inp[slot], inp_enable[slot] = sel, ENABLE

    out = {o: OutSel.ALU_OUT for o in OutPath}
    out[OutPath.WR0_LO] = OutSel.DELAY_0
    out_enable = {o: DISABLE for o in OutPath}
    out_enable[OutPath.WR0_LO] = ENABLE
    out_enable[OutPath.WR0_HI] = ENABLE

    return [
        UopConfig(
            datapath_config=stages,
            inp=inp,
            inp_enable=inp_enable,
            out=out,
            out_enable=out_enable,
            accum_enabled=DISABLE,
            require_inp0=1,
            require_inp1=1,
            trigger=(Trigger.SRC_TENSOR_DONE, Trigger.NONE, Trigger.NONE),
            next_uop=(0, 0, 0),
            repeat_count=0,
        )
    ]


def _register_gat_op():
    """Register the fused score op (1x + 2x_1p programs) in dve_ops.OPS."""
    from concourse import dve_ops
    from concourse.dve_spec import Spec, Src0, Src1, C0, C1, maxx, lower, _has_src1
    from concourse.dve_uop import DveOpSpec

    for op in dve_ops.OPS:
        if op.name == _GAT_NAME:
            return op

    spec = Spec(
        body=Src0 * maxx(Src1 * C0, C1),
        reference=lambda in0, in1, s0, s1, imm2: (
            in0.astype(np.float32) * np.maximum(in1.astype(np.float32) * s0, s1)
        ).astype(np.float32),
    )
    row = 1 + len(dve_ops.OPS)
    assert row < 0x20
    dve_ops._SUB_OPCODE_FOR_NAME[_GAT_NAME] = row

    shas = {}
    for ver in ("v3", "v4"):
        enriched = DveOpSpec(
            name=_GAT_NAME,
            opcode=row,
            uops=lower(spec, ver=ver),
            uops_2x=_build_2x_uops(ver),
            perf_max=1,
            rd1_en=_has_src1(spec),
        )
        enriched.validate(ver)
        shas[ver] = enriched.sha(ver)
        # Pre-seed the compile cache so DveOp.compile() returns the spec
        # enriched with the 2x program.
        dve_ops._COMPILE_CACHE[(_GAT_NAME, ver)] = enriched

    op = dve_ops.DveOp(_GAT_NAME, spec, subdim=False, uops_sha=shas)
    dve_ops.OPS.append(op)
    dve_ops.CUSTOM_DVE_SPECS[_GAT_NAME] = spec
    return op


def _build_program():
    import concourse.bass as bass
    import concourse.mybir as mybir
    import concourse.tile as tile
    from concourse import bacc
    from concourse.masks import make_identity
    from contextlib import ExitStack

    gat_op = _register_gat_op()

    f32 = mybir.dt.float32
    bf16 = mybir.dt.bfloat16
    Alu = mybir.AluOpType
    Act = mybir.ActivationFunctionType

    nc = bacc.Bacc("TRN2", target_bir_lowering=False, debug=False)
    atp = nc.declare_dram_parameter("atp", [R, 128, NT * IS], bf16, isOutput=False)
    whp = nc.declare_dram_parameter("whp", [R, 128, NT * HW], bf16, isOutput=False)
    e2i = nc.declare_dram_parameter("e2i", [R, 128, Hh * IS], bf16, isOutput=False)
    ejc = nc.declare_dram_parameter("ejc", [R, 128, 2 * Hh * NT], f32, isOutput=False)
    hres = nc.declare_dram_parameter("hres", [NIQ, 128, D], f32, isOutput=False)
    gmb = nc.declare_dram_parameter("gmb", [2, 128, D], f32, isOutput=False)
    out = nc.declare_dram_parameter("out", [NIQ, 128, D], f32, isOutput=True)

    with ExitStack() as ctx:
        tc = ctx.enter_context(tile.TileContext(nc))
        const = ctx.enter_context(tc.tile_pool(name="const", bufs=1))
        atp_pool = ctx.enter_context(tc.tile_pool(name="atp", bufs=2))
        u_pool = ctx.enter_context(tc.tile_pool(name="u", bufs=6))
        aggsb_pool = ctx.enter_context(tc.tile_pool(name="aggsb", bufs=4))
        tq_pool = ctx.enter_context(tc.tile_pool(name="tq", bufs=2))
        small = ctx.enter_context(tc.tile_pool(name="small", bufs=4))
        epi_pool = ctx.enter_context(tc.tile_pool(name="epi", bufs=2))
        psum_agg = ctx.enter_context(tc.tile_pool(name="pagg", bufs=1, space="PSUM"))
        psum_tp = ctx.enter_context(tc.tile_pool(name="ptp", bufs=4, space="PSUM"))

        # ---- constants / per-relation operands ----
        # The first score op needs atp[0] chunk 0 + e2i[0] + ejc[0]: those go
        # first on the otherwise-empty SP (sync) DGE queue, with a tiny first
        # chunk so its transfer lands ASAP.  Everything else is issued on
        # GpSimd/ACT queues; r=1,2 operands are prefetched inside the r loop.
        CHUNK_PLANS = {0: (1, 3, 4, 4, 4), 1: (4, 4, 4, 4), 2: (4, 4, 4, 4)}

        def issue_atp(eng, r):
            chunks, jt0 = [], 0
            for njt in CHUNK_PLANS[r]:
                a_g = atp_pool.tile([128, njt * IS], bf16, tag=f"atp{jt0}_{njt}")
                eng.dma_start(
                    a_g[:], atp[r][:, jt0 * IS : (jt0 + njt) * IS]
                )
                for jl in range(njt):
                    chunks.append((a_g, jl))
                jt0 += njt
            return chunks

        whp_sb, e2i_sb, ej1_sb, ej2_sb = [None] * R, [None] * R, [None] * R, [None] * R

        def issue_consts(eng_ee, eng_w, r):
            e = const.tile([128, Hh * IS], bf16, tag=f"e2i{r}")
            eng_ee.dma_start(e[:], e2i[r])
            e2i_sb[r] = e
            d = const.tile([128, 2 * Hh * NT], f32, tag=f"ejc{r}")
            eng_ee.dma_start(d[:], ejc[r])
            ej1_sb[r] = d[:, 0 : Hh * NT]
            ej2_sb[r] = d[:, Hh * NT : 2 * Hh * NT]
            w = const.tile([128, NT * HW], bf16, tag=f"whp{r}")
            eng_w.dma_start(w[:], whp[r])
            whp_sb[r] = w

        with tc.high_priority():
            # only three transfers gate the first score op: keep the ACT DGE
            # queue exclusively for them; everything else goes via GpSimd
            a0_first = atp_pool.tile([128, 1 * IS], bf16, tag="atp0_first")
            nc.scalar.dma_start(a0_first[:], atp[0][:, 0:IS])
            e0 = const.tile([128, Hh * IS], bf16, tag="e2i0")
            nc.scalar.dma_start(e0[:, 0:IS], e2i[0][:, 0:IS])
            d0 = const.tile([128, 2 * Hh * NT], f32, tag="ejc0")
            nc.scalar.dma_start(d0[:], ejc[0])
            nc.gpsimd.dma_start(e0[:, IS : Hh * IS], e2i[0][:, IS : Hh * IS])
            e2i_sb[0] = e0
            ej1_sb[0] = d0[:, 0 : Hh * NT]
            ej2_sb[0] = d0[:, Hh * NT : 2 * Hh * NT]
            w0 = const.tile([128, NT * HW], bf16, tag="whp0")
            nc.gpsimd.dma_start(w0[:], whp[0])
            whp_sb[0] = w0

            a_chunks = {0: [(a0_first, 0)]}
            jt0 = 1
            for njt in CHUNK_PLANS[0][1:]:
                a_g = atp_pool.tile([128, njt * IS], bf16, tag=f"atp{jt0}_{njt}")
                nc.gpsimd.dma_start(a_g[:], atp[0][:, jt0 * IS : (jt0 + njt) * IS])
                for jl in range(njt):
                    a_chunks[0].append((a_g, jl))
                jt0 += njt

        ident = const.tile([128, 128], f32, tag="ident")
        make_identity(nc, ident[:])

        hres_sb, acc = [], []
        for t in range(NIQ):
            hh = const.tile([128, D], f32, tag=f"hres{t}")
            nc.gpsimd.dma_start(hh[:], hres[t])
            hres_sb.append(hh)
            acc_t = const.tile([128, D], f32, tag=f"acc{t}", name=f"acc{t}")
            acc.append(acc_t)
        gam = const.tile([128, D], f32, tag="gam")
        nc.gpsimd.dma_start(gam[:], gmb[0])
        bet = const.tile([128, D], f32, tag="bet")
        nc.gpsimd.dma_start(bet[:], gmb[1])
        eps_b = const.tile([128, 1], f32, tag="eps_b")
        nc.gpsimd.memset(eps_b[:], LN_EPS)

        # ---- hot loop over relations ----
        for r in range(R):
            if r == 0:
                deferred = []
            # prefetch r+1's operands while r computes
            if r + 1 < R:
                issue_consts(nc.gpsimd, nc.gpsimd, r + 1)
                a_chunks[r + 1] = issue_atp(nc.scalar, r + 1)
            a_sb = a_chunks.pop(r)

            aggp = [
                psum_agg.tile([33, IS], f32, tag=f"agg{h}", name=f"agg{h}")
                for h in range(Hh)
            ]

            for jt in range(NT):
                if jt == 4 and deferred:
                    # flush the PREVIOUS relation's normalize chain now: its
                    # transposes finished long ago, so the vector reciprocal
                    # issues without stalling the GAT stream (emitting it at
                    # the relation boundary made Vector wait on fresh
                    # transposes before starting this relation's scores)
                    for fn in deferred:
                        fn()
                    deferred = []
                a_g, jl = a_sb[jt]
                u = u_pool.tile([128, Hh * IS], bf16, tag="u")
                for h in range(Hh):
                    bi = nc.vector._custom_dve(
                        gat_op,
                        out=u[:, h * IS : (h + 1) * IS],
                        in0=a_g[:, jl * IS : (jl + 1) * IS],
                        in1=e2i_sb[r][:, h * IS : (h + 1) * IS],
                        s0=ej2_sb[r][:, h * NT + jt : h * NT + jt + 1],
                        s1=ej1_sb[r][:, h * NT + jt : h * NT + jt + 1],
                    )
                    # byte-36[7:6]: allow the engine to select the 2X_1PORT
                    # uop program (it falls back to 1x if the mem-pattern
                    # doesn't qualify)
                    bi.ins.perf_max = 1
                for h in range(Hh):
                    nc.tensor.matmul(
                        aggp[h][:, :],
                        lhsT=whp_sb[r][:, jt * HW + h * 33 : jt * HW + (h + 1) * 33],
                        rhs=u[:, h * IS : (h + 1) * IS],
                        start=(jt == 0),
                        stop=(jt == NT - 1),
                    )

            # ---- per (r, h): normalize by R*row-sums, accumulate over r ----
            # The PSUM->SBUF copy and transposes are emitted now (the psum_agg
            # pool needs them before the next relation's matmuls), but the
            # reciprocal/contrib/accumulate chain is deferred into the next
            # relation's score stream.
            for h in range(Hh):
                asb = aggsb_pool.tile([33, IS], f32, tag="aggsb")
                nc.scalar.copy(asb[:], aggp[h][:])
                tp = psum_tp.tile([128, NIQ * 33], f32, tag="tp")
                for it in range(NIQ):
                    nc.tensor.transpose(
                        tp[:, it * 33 : (it + 1) * 33],
                        asb[:, it * 128 : (it + 1) * 128],
                        ident[:33, :33],
                    )
                last = r == R - 1

                def finish(r=r, h=h, tp=tp, last=last):
                    rec = small.tile([128, NIQ], f32, tag="rec")
                    nc.vector.reciprocal(rec[:], tp[:, 32 : NIQ * 33 : 33])
                    for it in range(NIQ):
                        dst = acc[it][:, h * hd : (h + 1) * hd]
                        contrib = small.tile([128, hd], f32, tag="contrib")
                        if last:
                            # drain phase: Vector is idle — keep the chain on
                            # Vector to avoid cross-engine semaphore latency
                            nc.vector.tensor_scalar(
                                out=contrib[:],
                                in0=tp[:, it * 33 : it * 33 + 32],
                                scalar1=rec[:, it : it + 1],
                                scalar2=None,
                                op0=Alu.mult,
                            )
                            nc.vector.tensor_add(dst, dst, contrib[:])
                            continue
                        nc.scalar.activation(
                            contrib[:],
                            tp[:, it * 33 : it * 33 + 32],
                            Act.Copy,
                            scale=rec[:, it : it + 1],
                        )
                        if r == 0:
                            nc.gpsimd.tensor_copy(dst, contrib[:])
                        else:
                            nc.gpsimd.tensor_add(dst, dst, contrib[:])

                if last:
                    finish()
                else:
                    deferred.append(finish)

        # ---- epilogue: residual + LayerNorm (runs in the tail; Vector is
        # idle there, so the big elementwise ops go back on Vector) ----
        for t in range(NIQ):
            x = epi_pool.tile([128, D], f32, tag="x")
            nc.vector.tensor_add(x[:], acc[t][:], hres_sb[t][:])
            xcp = epi_pool.tile([128, D], f32, tag="xcp")
            mu_r = small.tile([128, 1], f32, tag="mu_r")
            nc.scalar.activation(xcp[:], x[:], Act.Copy, accum_out=mu_r[:])
            mu = small.tile([128, 1], f32, tag="mu")
            nc.scalar.activation(mu[:], mu_r[:], Act.Copy, scale=1.0 / D)
            xc = epi_pool.tile([128, D], f32, tag="xc")
            nc.vector.tensor_scalar(
                out=xc[:], in0=x[:], scalar1=mu[:], scalar2=None, op0=Alu.subtract
            )
            sq = epi_pool.tile([128, D], f32, tag="sq")
            vs_r = small.tile([128, 1], f32, tag="vs_r")
            nc.scalar.activation(sq[:], xc[:], Act.Square, accum_out=vs_r[:])
            std = small.tile([128, 1], f32, tag="std")
            nc.scalar.activation(
                std[:], vs_r[:], Act.Sqrt, scale=1.0 / D, bias=eps_b[:]
            )
            rstd = small.tile([128, 1], f32, tag="rstd")
            nc.vector.reciprocal(rstd[:], std[:])
            xn = epi_pool.tile([128, D], f32, tag="xn")
            nc.vector.tensor_scalar(
                out=xn[:], in0=xc[:], scalar1=rstd[:], scalar2=None, op0=Alu.mult
            )
            xg = epi_pool.tile([128, D], f32, tag="xg")
            nc.vector.tensor_mul(xg[:], xn[:], gam[:])
            xo = epi_pool.tile([128, D], f32, tag="xo")
            nc.vector.tensor_add(xo[:], xg[:], bet[:])
            nc.sync.dma_start(out[t], xo[:])

    nc.compile()
    return nc


def _host_pack(H, A, W, a_src, a_dst, ln_gamma, ln_beta):
    H = np.asarray(H, np.float32)
    A = np.asarray(A)
    W = np.asarray(W, np.float32)
    a_src = np.asarray(a_src, np.float32)
    a_dst = np.asarray(a_dst, np.float32)
    ln_gamma = np.asarray(ln_gamma, np.float32)
    ln_beta = np.asarray(ln_beta, np.float32)

    Hm = H.reshape(B * N, D)
    # Wh[r,b,n,h,f]
    Wh = np.empty((R, B, N, Hh, hd), np.float32)
    for r in range(R):
        for h in range(Hh):
            Wh[r, :, :, h, :] = (Hm @ W[r, h]).reshape(B, N, hd)
    es = np.einsum("rbnhf,rhf->rbhn", Wh, a_src)  # [R,B,Hh,N]
    ed = np.einsum("rbnhf,rhf->rbhn", Wh, a_dst)

    # packed Wh + R column, [R, B, 128, NT*132] bf16; the R column makes the
    # matmul emit R*rowsum so 1/(R*rowsum) handles softmax + mean-over-R.
    whp = np.full((R, B, NT, 128, Hh, 33), float(R), np.float32)
    whp[:, :, :, :, :, :32] = Wh.transpose(0, 1, 2, 3, 4).reshape(
        R, B, NT, 128, Hh, hd
    )
    whp = (
        whp.reshape(R, B, NT, 128, HW)
        .transpose(0, 1, 3, 2, 4)
        .reshape(R, B, 128, NT * HW)
        .astype(ml_dtypes.bfloat16)
    )

    # E1j = exp(ed), E2j = exp(0.2 ed): [R, B, 128, 2*Hh*NT] f32
    edr = ed.reshape(R, B, Hh, NT, 128).transpose(0, 1, 4, 2, 3)  # [R,B,128,Hh,NT]
    ejc = (
        np.concatenate([np.exp(edr), np.exp(0.2 * edr)], axis=3)
        .reshape(R, B, 128, 2 * Hh * NT)
        .astype(np.float32)
    )
    ejc = np.ascontiguousarray(ejc)

    # mask, transposed, raw {0,1} bf16: At[r,b,j,i]
    At = A.transpose(0, 1, 3, 2)
    atp_full = At.astype(ml_dtypes.bfloat16).reshape(R, B, NT, 128, N)

    gmbase = np.stack(
        [
            np.broadcast_to(ln_gamma, (128, D)),
            np.broadcast_to(ln_beta, (128, D)),
        ]
    ).astype(np.float32)
    gmbase = np.ascontiguousarray(gmbase)

    # E2i = exp(-0.8 es) broadcast rows, per-core slices below
    e2i_full = np.exp(-0.8 * es)  # [R,B,Hh,N]

    in_maps = []
    for c in range(NCORES):
        b, iq = divmod(c, NIQ)
        i0 = iq * IS
        atp_c = np.ascontiguousarray(
            atp_full[:, b, :, :, i0 : i0 + IS].transpose(0, 2, 1, 3)
        ).reshape(R, 128, NT * IS)
        e2i_c = (
            e2i_full[:, b, :, i0 : i0 + IS]
            .reshape(R, Hh * IS)
            .astype(ml_dtypes.bfloat16)
        )
        e2i_c = np.ascontiguousarray(
            np.broadcast_to(e2i_c[:, None, :], (R, 128, Hh * IS))
        )
        hres_c = np.ascontiguousarray(H[b, i0 : i0 + IS, :]).reshape(NIQ, 128, D)
        in_maps.append(
            {
                "atp": atp_c,
                "whp": np.ascontiguousarray(whp[:, b]),
                "e2i": e2i_c,
                "ejc": np.ascontiguousarray(ejc[:, b]),
                "hres": hres_c,
                "gmb": gmbase,
            }
        )
    return in_maps


def kernel(H, A, W, a_src, a_dst, ln_gamma, ln_beta):
    from concourse.bass_utils import run_bass_kernel_spmd

    if "nc" not in _CACHE:
        _CACHE["nc"] = _build_program()
    nc = _CACHE["nc"]

    in_maps = _host_pack(H, A, W, a_src, a_dst, ln_gamma, ln_beta)
    res = run_bass_kernel_spmd(nc, in_maps, list(range(NCORES)))

    full = np.empty((B, N, D), np.float32)
    for c in range(NCORES):
        b, iq = divmod(c, NIQ)
        i0 = iq * IS
        o = np.asarray(res.results[c]["out"], np.float32).reshape(IS, D)
        full[b, i0 : i0 + IS, :] = o
    return full
